# revision 13
# baseline (speedup 1.0000x reference)
"""MARN (multi-attention recurrent network) Trainium2 Bass kernel.

Strategy v2 (two-group software pipeline):
  - Data-parallel over batch N=2048 across 8 cores (256 rows each).
  - Feature-major on-chip layout: features on SBUF partitions, batch on the
    free dim.
  - The per-core batch of 256 is split into TWO independent groups of 128.
    The T-recurrence chain (gates -> acts -> cell -> attention -> softmax ->
    MLP -> z) is latency-bound; two groups give every engine a second
    independent stream to chew on while the other group's chain waits.
  - All matmul operands are bf16 (1 cyc/row at any moving size; fp32r would
    drop to 4 cyc/row below N=256). PSUM accumulation stays fp32.
  - Sigmoid via 0.5*tanh(0.5x)+0.5 so everything stays on the single
    `exp_and_others` ACT table (no 2.7us table reloads). States scaled 2x
    (C=2c, H=2h) with the 0.5 folded into consuming weights host-side.
  - Softmax over the feature dim via ones-matmuls; exp(att_b) folded into
    sum + reduction weights.
  - x-projections (no recurrence dependency) run at N=256 over both groups
    into a shared gate psum, prefetched one step ahead to fill PE bubbles.
"""
import sys

if "/opt/trn_rl_repo" not in sys.path:
    sys.path.insert(0, "/opt/trn_rl_repo")

import numpy as np
import ml_dtypes

import concourse.bass as bass
import concourse.bacc as bacc
import concourse.tile as tile
from concourse import mybir
from concourse.bass_utils import run_bass_kernel_spmd

F32 = mybir.dt.float32
F32R = mybir.dt.float32r
BF16 = mybir.dt.bfloat16
AF = mybir.ActivationFunctionType
ALU = mybir.AluOpType
BD = ml_dtypes.bfloat16

D_L, D_A, D_V = 300, 74, 35
DH_L, DH_A, DH_V = 128, 32, 32
H = DH_L + DH_A + DH_V          # 192
K = 4
RL, RA, RV = 32, 16, 16
RED = RL + RA + RV              # 64
MAP_H = 256
H_OUT = 64
T, N = 128, 2048
D = D_L + D_A + D_V             # 409
NCORES = 8
B = N // NCORES                 # 256 per-core batch
G = 2
BG = B // G                     # 128 per group


def r(x):
    return x.bitcast(F32R)


# ----------------------------------------------------------------- host pack
def pack_weights(i):
    """Pack reference weights into the on-chip layouts. Returns dict name->np."""
    w = {}
    f32 = np.float32

    # ---- L cell ----
    WlT = i["Wl_w"].T.astype(f32)                      # [300, 512]
    wlT = np.zeros((3, 128, 4 * DH_L), f32)
    wlT.reshape(384, 512)[:300] = WlT
    w["wlT"] = wlT
    w["ulT"] = (0.5 * i["Ul_w"].T).astype(f32)         # [128, 512]
    VlT = i["Vl_w"].T.astype(f32)                      # [192, 512]
    w["vlT1"] = VlT[:128].copy()
    vlSt = np.zeros((128, 512), f32)
    vlSt[64:128] = VlT[128:192]
    w["vlSt"] = vlSt
    bl = (i["Wl_b"] + i["Ul_b"] + i["Vl_b"]).astype(f32)   # [512]
    blb = np.empty((128, 4), f32)
    for m in range(3):
        blb[:, m] = 0.5 * bl[m * 128:(m + 1) * 128]
    blb[:, 3] = bl[384:512]
    w["blb"] = blb

    # ---- A/V cells fused: gate chunk m in {f,i,o,g}, cols [a(32) | v(32)] ----
    Wa, Wv = i["Wa_w"].astype(f32), i["Wv_w"].astype(f32)   # [128,74],[128,35]
    Ua, Uv = i["Ua_w"].astype(f32), i["Uv_w"].astype(f32)   # [128,32]
    Va, Vv = i["Va_w"].astype(f32), i["Vv_w"].astype(f32)   # [128,192]
    GATES = ((0, 3), (2, 1))          # (lo-half gate, hi-half gate) per chunk
    wavT = np.zeros((128, 2, 128), f32)
    uavSt = np.zeros((128, 2, 128), f32)
    vavT1 = np.zeros((128, 2, 128), f32)
    for m, pair in enumerate(GATES):
        for h, g in enumerate(pair):
            ga = slice(g * 32, (g + 1) * 32)
            c0 = h * 64
            wavT[0:74, m, c0:c0 + 32] = Wa[ga].T
            wavT[74:109, m, c0 + 32:c0 + 64] = Wv[ga].T
            uavSt[0:32, m, c0:c0 + 32] = 0.5 * Ua[ga].T
            uavSt[32:64, m, c0 + 32:c0 + 64] = 0.5 * Uv[ga].T
            uavSt[64:128, m, c0:c0 + 32] = Va[ga, 128:192].T
            uavSt[64:128, m, c0 + 32:c0 + 64] = Vv[ga, 128:192].T
            vavT1[:, m, c0:c0 + 32] = Va[ga, 0:128].T
            vavT1[:, m, c0 + 32:c0 + 64] = Vv[ga, 0:128].T
    w["wavT"], w["uavSt"], w["vavT1"] = wavT, uavSt, vavT1
    ba = (i["Wa_b"] + i["Ua_b"] + i["Va_b"]).astype(f32)
    bv = (i["Wv_b"] + i["Uv_b"] + i["Vv_b"]).astype(f32)
    bav = np.empty((128, 2), f32)
    for m, pair in enumerate(GATES):
        for h, g in enumerate(pair):
            s = 0.5 if g < 3 else 1.0
            bav[h * 64:h * 64 + 32, m] = s * ba[g * 32:(g + 1) * 32]
            bav[h * 64 + 32:h * 64 + 64, m] = s * bv[g * 32:(g + 1) * 32]
    w["bav"] = bav

    # ---- attention ----
    attT = i["att_w"].T.astype(f32)                    # [192, 768] cols (k,h')
    attT1 = np.empty((128, 4, 192), f32)
    attT2 = np.empty((64, 4, 192), f32)
    for k in range(4):
        blk = 0.5 * attT[:, k * 192:(k + 1) * 192]     # 0.5: C=2c fold
        attT1[:, k, :] = blk[0:128]
        attT2[:, k, :] = blk[128:192]
    w["attT1"], w["attT2"] = attT1, attT2
    eb = np.exp(i["att_b"].astype(np.float64)).astype(f32)  # [768]
    expb1 = np.stack([eb[k * 192:k * 192 + 128] for k in range(4)], 1)   # [128,4]
    expb2 = np.stack([eb[k * 192 + 128:(k + 1) * 192] for k in range(4)], 1)  # [64,4]

    ebZ1 = np.zeros((128, 4, 4), f32)
    ebZ2 = np.zeros((64, 4, 4), f32)
    for k in range(4):
        ebZ1[:, k, k] = expb1[:, k]
        ebZ2[:, k, k] = expb2[:, k]
    w["expbZ1"], w["expbZ2"] = ebZ1, ebZ2
    selm1 = np.zeros((4, 4, 128), f32)
    for k in range(4):
        selm1[k, k, :] = 1.0
    w["selm1"] = selm1

    # ---- reductions, with 0.5*exp(att_b) folded in ----
    rl = i["rl_w"].astype(f32)                         # [32, 512] cols (k,h')
    rlT = np.empty((128, 4, 32), f32)
    for k in range(4):
        scale = 0.5 * eb[k * 192:k * 192 + 128]        # [128]
        rlT[:, k, :] = rl[:, k * 128:(k + 1) * 128].T * scale[:, None]
    w["rlT"] = rlT
    ra = i["ra_w"].astype(f32)                         # [16, 128]
    rv = i["rv_w"].astype(f32)                         # [16, 128]
    ravT = np.zeros((64, 4, 64), f32)
    for k in range(4):
        sa = 0.5 * eb[k * 192 + 128:k * 192 + 160]     # [32]
        sv = 0.5 * eb[k * 192 + 160:k * 192 + 192]     # [32]
        ravT[0:32, k, 32:48] = ra[:, k * 32:(k + 1) * 32].T * sa[:, None]
        ravT[32:64, k, 48:64] = rv[:, k * 32:(k + 1) * 32].T * sv[:, None]
    w["ravT"] = ravT
    w["rbias"] = np.concatenate([i["rl_b"], i["ra_b"], i["rv_b"]]).astype(f32)[:, None]  # [64,1]

    # ---- z MLP ----
    w["fc1T"] = i["fc1_w"].T.astype(f32)               # [64, 256]
    fc1b = np.empty((128, 2), f32)
    fc1b[:, 0] = i["fc1_b"][0:128]
    fc1b[:, 1] = i["fc1_b"][128:256]
    w["fc1b"] = fc1b
    fc2T = np.empty((128, 2, 192), f32)
    fc2wT = i["fc2_w"].T.astype(f32)                   # [256, 192]
    fc2T[:, 0, :] = fc2wT[0:128]
    fc2T[:, 1, :] = fc2wT[128:256]
    w["fc2T"] = fc2T
    w["fc2b1"] = i["fc2_b"].astype(f32)[0:128, None]   # [128,1]
    w["fc2b2"] = i["fc2_b"].astype(f32)[128:192, None]  # [64,1]

    # ---- output head (h parts scaled 0.5 to consume H=2h) ----
    o1T = i["o1_w"].T.astype(f32)                      # [384, 64]
    w["o1T0"] = (0.5 * o1T[0:128]).copy()
    o1T1 = np.empty((128, 64), f32)
    o1T1[0:64] = 0.5 * o1T[128:192]                    # H_av
    o1T1[64:128] = o1T[320:384]                        # z2
    w["o1T1"] = o1T1
    w["o1T2"] = o1T[192:320].copy()                    # z1
    w["o1b"] = i["o1_b"].astype(f32)[:, None]          # [64,1]
    w["o2T"] = i["o2_w"].T.astype(f32)                 # [64, 1]
    w["o2bt"] = np.asarray(i["o2_b"], f32).reshape(1, 1)
    return w


F32_WEIGHTS = {"blb", "bav", "rbias", "fc1b", "fc2b1", "fc2b2", "o1b", "o2bt"}

WEIGHT_SHAPES = {
    "wlT": (3, 128, 512), "ulT": (128, 512), "vlT1": (128, 512),
    "vlSt": (128, 512), "blb": (128, 4),
    "wavT": (128, 2, 128), "uavSt": (128, 2, 128), "vavT1": (128, 2, 128),
    "bav": (128, 2),
    "attT1": (128, 4, 192), "attT2": (64, 4, 192),
    "expbZ1": (128, 4, 4), "expbZ2": (64, 4, 4),
    "selm1": (4, 4, 128),
    "rlT": (128, 4, 32), "ravT": (64, 4, 64), "rbias": (64, 1),
    "fc1T": (64, 256), "fc1b": (128, 2),
    "fc2T": (128, 2, 192), "fc2b1": (128, 1), "fc2b2": (64, 1),
    "o1T0": (128, 64), "o1T1": (128, 64), "o1T2": (128, 64),
    "o1b": (64, 1), "o2T": (64, 1), "o2bt": (1, 1),
}


# ------------------------------------------------------------------ program
def build_nc(t_steps=T, x_bufs=3):
    nc = bacc.Bacc("TRN2", target_bir_lowering=False, debug=False,
                   num_devices=NCORES)
    # x pre-packed host-side to the SBUF tile layout [128 part, 4 slots, B]:
    # slots 0/1 = x_l[0:128]/[128:256], slot 2 rows 0:44 = x_l[256:300],
    # slot 3 rows 0:109 = x_a|x_v. One contiguous 2KB-per-partition DMA/step.
    xd = nc.dram_tensor("xT", [t_steps, 128, 4 * B], BF16, kind="ExternalInput").ap()
    out_d = nc.dram_tensor("out", [1, B], F32, kind="ExternalOutput").ap()
    wd = {n: nc.dram_tensor(n, list(s), F32 if n in F32_WEIGHTS else BF16,
                            kind="ExternalInput").ap()
          for n, s in WEIGHT_SHAPES.items()}

    with nc.allow_low_precision(reason="bf16 operand kernel; psum accumulation "
                                "remains fp32"):
        with tile.TileContext(nc) as tc:
            _emit(tc, xd, out_d, wd, t_steps, x_bufs)
    nc.compile()
    return nc


def _emit(tc, xd, out_d, wd, t_steps, x_bufs):
    nc = tc.nc
    import contextlib
    ctx = contextlib.ExitStack()

    wp = ctx.enter_context(tc.tile_pool(name="weights", bufs=1))
    sp = ctx.enter_context(tc.tile_pool(name="state", bufs=1))
    xp = ctx.enter_context(tc.tile_pool(name="x", bufs=x_bufs))
    tp = ctx.enter_context(tc.tile_pool(name="work", bufs=2))
    ep = ctx.enter_context(tc.tile_pool(name="exps", bufs=2))
    # PSUM: pGATE [128,6,256]f32 = 6KB = banks 0-2 (shared, bufs=1);
    # per-group work rings 2 x 2KB-slots = 1+1 banks each.
    pGATE = ctx.enter_context(tc.tile_pool(name="pGATE", bufs=1, space="PSUM"))
    pW = [ctx.enter_context(tc.tile_pool(name=f"pW{g}", bufs=2, space="PSUM"))
          for g in range(G)]

    # ---- persistent weights ----
    W = {}
    for n, shape in WEIGHT_SHAPES.items():
        tl = wp.tile(list(shape) if n != "wlT" else [128, 3, 512],
                     F32 if n in F32_WEIGHTS else BF16, tag=n)
        if n == "wlT":
            nc.sync.dma_start(tl[:], wd[n].rearrange("j p m -> p j m"))
        else:
            nc.sync.dma_start(tl[:], wd[n][:])
        W[n] = tl

    # ---- per-group states (in-place updated each step) ----
    Hl, St, Z1, Cl, Cav = [], [], [], [], []
    for g in range(G):
        Hl.append(sp.tile([128, BG], BF16, tag=f"Hl{g}", name=f"Hl{g}"))
        St.append(sp.tile([128, BG], BF16, tag=f"St{g}", name=f"St{g}"))
        Z1.append(sp.tile([128, BG], BF16, tag=f"Z1{g}", name=f"Z1{g}"))
        Cl.append(sp.tile([128, BG], BF16, tag=f"Cl{g}", name=f"Cl{g}"))
        Cav.append(sp.tile([64, BG], BF16, tag=f"Cav{g}", name=f"Cav{g}"))
        for s in (Hl[g], St[g], Z1[g], Cl[g]):
            nc.vector.memset(s[:].bitcast(F32), 0.0)
        nc.vector.memset(Cav[g][:].bitcast(F32), 0.0)

    xts = {}

    def load_x(t):
        xt = xp.tile([128, 4, B], BF16, tag="xt")
        nc.sync.dma_start(xt[:], xd[t, :, :])
        xts[t] = xt

    def x_mms(t):
        """Input projections for step t at N=256 (both groups), into a fresh
        shared gate psum GP [128, 6, 256]: slots 0-3 L gate chunks (banks 0-1),
        slots 4-5 AV chunks (bank 2). One start per bank."""
        GP = pGATE.tile([128, 6, B], F32, tag="gate")
        xt = xts[t]
        for m in range(4):
            for j in range(3):
                kk = 128 if j < 2 else 44
                nc.tensor.matmul(GP[:, m, :], W["wlT"][0:kk, j, m * 128:(m + 1) * 128],
                                 xt[0:kk, j, :], start=(j == 0 and m % 2 == 0),
                                 stop=False)
        for m in range(2):
            nc.tensor.matmul(GP[:, 4 + m, :], W["wavT"][0:109, m, :], xt[0:109, 3, :],
                             start=(m == 0), stop=False)
        return GP

    gcols = [slice(g * BG, (g + 1) * BG) for g in range(G)]

    import os
    AFENCE = int(os.environ.get("KAFENCE", "9"))

    def emit_A(g, t, GP):
        """Gate matmuls + activations + cell updates for group g, step t."""
        gc = gcols[g]
        if AFENCE < 1:
            return
        for m in range(4):
            nc.tensor.matmul(GP[:, m, gc], W["ulT"][:, m * 128:(m + 1) * 128],
                             Hl[g][:], start=False, stop=False)
        for m in range(4):
            ms = slice(m * 128, (m + 1) * 128)
            nc.tensor.matmul(GP[:, m, gc], W["vlT1"][:, ms], Z1[g][:],
                             start=False, stop=False)
            nc.tensor.matmul(GP[:, m, gc], W["vlSt"][:, ms], St[g][:],
                             start=False, stop=(g == G - 1 and m % 2 == 1))
        for m in range(2):
            nc.tensor.matmul(GP[:, 4 + m, gc], W["uavSt"][:, m, :], St[g][:],
                             start=False, stop=False)
            nc.tensor.matmul(GP[:, 4 + m, gc], W["vavT1"][:, m, :], Z1[g][:],
                             start=False, stop=(g == G - 1 and m == 1))
        if AFENCE < 2:
            return
        # gate activations: tf/ti/to = tanh(0.5 s + 0.5 b), tg = tanh(s + b)
        tf = tp.tile([128, 4, BG], BF16, tag=f"tfl{g}", name=f"tfl{g}")
        for m in range(4):
            sc = 0.5 if m < 3 else 1.0
            nc.scalar.activation(tf[:, m, :], GP[:, m, gc], AF.Tanh,
                                 bias=W["blb"][:, m:m + 1], scale=sc)
        ta = tp.tile([128, 2, BG], BF16, tag=f"tav{g}", name=f"tav{g}")
        nc.scalar.activation(ta[0:64, 0, :], GP[0:64, 4, gc], AF.Tanh,
                             bias=W["bav"][0:64, 0:1], scale=0.5)
        nc.scalar.activation(ta[64:128, 0, :], GP[64:128, 4, gc], AF.Tanh,
                             bias=W["bav"][64:128, 0:1], scale=1.0)
        nc.scalar.activation(ta[:, 1, :], GP[:, 5, gc], AF.Tanh,
                             bias=W["bav"][:, 1:2], scale=0.5)
        if AFENCE < 3:
            return
        # cell updates (C=2c, H=2h)
        s1l = tp.tile([128, BG], BF16, tag=f"s1l{g}", name=f"s1l{g}")
        s2l = tp.tile([128, BG], BF16, tag=f"s2l{g}", name=f"s2l{g}")
        nc.vector.scalar_tensor_tensor(s1l[:], tf[:, 0, :], 1.0, Cl[g][:], ALU.add, ALU.mult)
        nc.vector.scalar_tensor_tensor(s2l[:], tf[:, 1, :], 1.0, tf[:, 3, :], ALU.add, ALU.mult)
        nc.vector.scalar_tensor_tensor(Cl[g][:], s1l[:], 0.5, s2l[:], ALU.mult, ALU.add)
        s1a = tp.tile([128, BG], BF16, tag=f"s1a{g}", name=f"s1a{g}")
        s2a = tp.tile([128, BG], BF16, tag=f"s2a{g}", name=f"s2a{g}")
        nc.vector.scalar_tensor_tensor(s1a[64:128, :], ta[0:64, 0, :], 1.0, Cav[g][:], ALU.add, ALU.mult)
        nc.vector.scalar_tensor_tensor(s2a[64:128, :], ta[64:128, 1, :], 1.0, ta[64:128, 0, :], ALU.add, ALU.mult)
        nc.vector.scalar_tensor_tensor(Cav[g][:], s1a[64:128, :], 0.5, s2a[64:128, :], ALU.mult, ALU.add)
        tcl = tp.tile([128, BG], BF16, tag=f"tcl{g}", name=f"tcl{g}")
        tca = tp.tile([64, BG], BF16, tag=f"tca{g}", name=f"tca{g}")
        nc.scalar.activation(tcl[:], Cl[g][:], AF.Tanh, scale=0.5)
        nc.scalar.activation(tca[:], Cav[g][:], AF.Tanh, scale=0.5)
        nc.vector.scalar_tensor_tensor(Hl[g][:], tf[:, 2, :], 1.0, tcl[:], ALU.add, ALU.mult)
        nc.vector.scalar_tensor_tensor(St[g][0:64, :], ta[0:64, 1, :], 1.0, tca[:], ALU.add, ALU.mult)

    def emit_B1(g, t):
        """Attention matmuls, exp, softmax sums + reciprocal for group g."""
        att1 = pW[g].tile([128, 4, BG], F32, tag="w", name=f"att1_{g}")
        att2 = pW[g].tile([64, 4, BG], F32, tag="w", name=f"att2_{g}")
        for k in range(4):
            nc.tensor.matmul(att1[:, k, :], W["attT1"][:, k, 0:128], Cl[g][:],
                             start=(k == 0), stop=False)
            nc.tensor.matmul(att1[:, k, :], W["attT2"][:, k, 0:128], Cav[g][:],
                             start=False, stop=(k == 3))
            nc.tensor.matmul(att2[:, k, :], W["attT1"][:, k, 128:192], Cl[g][:],
                             start=(k == 0), stop=False)
            nc.tensor.matmul(att2[:, k, :], W["attT2"][:, k, 128:192], Cav[g][:],
                             start=False, stop=(k == 3))
        e1 = ep.tile([128, 4, BG], BF16, tag=f"e1{g}", name=f"e1{g}")
        e2 = ep.tile([64, 4, BG], BF16, tag=f"e2{g}", name=f"e2{g}")
        nc.scalar.activation(e1[:], att1[:], AF.Exp)
        nc.scalar.activation(e2[:], att2[:], AF.Exp)
        S4 = pW[g].tile([4, BG], F32, tag="w", name=f"S4_{g}")
        for ki in range(8):
            k, side = divmod(ki, 2)
            lh = W["expbZ1"][:, k, :] if side == 0 else W["expbZ2"][:, k, :]
            rh = e1[:, k, :] if side == 0 else e2[:, k, :]
            nc.tensor.matmul(S4[0:4, :], lh, rh, start=(ki == 0), stop=(ki == 7))
        rs4 = tp.tile([4, BG], BF16, tag=f"rs{g}", name=f"rs{g}")
        nc.vector.reciprocal(rs4[0:4, :], S4[0:4, :].bitcast(F32R))
        return e1, e2, rs4

    def emit_B2(g, t, e1, e2, rs4):
        """Broadcast 1/S, scale attended, reductions + z MLP for group g."""
        rb1 = pW[g].tile([128, 4, BG], F32, tag="w", name=f"rb1_{g}")
        for k in range(4):
            nc.tensor.matmul(rb1[:, k, :], W["selm1"][:, k, :], rs4[0:4, :],
                             start=(k == 0), stop=(k == 3))
        for k in range(4):
            nc.vector.tensor_tensor(e1[:, k, :], e1[:, k, :], Cl[g][:], ALU.mult)
            nc.vector.tensor_tensor(e1[:, k, :], e1[:, k, :],
                                    rb1[:, k, :].bitcast(F32R), ALU.mult)
            nc.vector.tensor_tensor(e2[:, k, :], e2[:, k, :], Cav[g][:], ALU.mult)
            nc.vector.tensor_tensor(e2[:, k, :], e2[:, k, :],
                                    rb1[0:64, k, :].bitcast(F32R), ALU.mult)
        redp = pW[g].tile([64, BG], F32, tag="w", name=f"redp_{g}")
        for k in range(3):
            nc.tensor.matmul(redp[0:64, :], W["ravT"][:, k, :], e2[:, k, :],
                             start=(k == 0), stop=False)
        for k in range(4):
            nc.tensor.matmul(redp[0:32, :], W["rlT"][:, k, :], e1[:, k, :],
                             start=False, stop=False)
        nc.tensor.matmul(redp[0:64, :], W["ravT"][:, 3, :], e2[:, 3, :],
                         start=False, stop=True)
        rsb = tp.tile([64, BG], BF16, tag=f"rsb{g}", name=f"rsb{g}")
        nc.scalar.activation(rsb[:], redp[:], AF.Identity, bias=W["rbias"][:])
        f1p = pW[g].tile([128, 2, BG], F32, tag="w", name=f"f1p_{g}")
        for m in range(2):
            nc.tensor.matmul(f1p[:, m, :], W["fc1T"][:, m * 128:(m + 1) * 128],
                             rsb[:], start=(m == 0), stop=(m == 1))
        zr = tp.tile([128, 2, BG], BF16, tag=f"zr{g}", name=f"zr{g}")
        for m in range(2):
            nc.scalar.activation(zr[:, m, :], f1p[:, m, :], AF.Relu,
                                 bias=W["fc1b"][:, m:m + 1])
        zp = pW[g].tile([128, 2, BG], F32, tag="w", name=f"zp_{g}")
        for j in range(2):
            nc.tensor.matmul(zp[:, 0, :], W["fc2T"][:, j, 0:128], zr[:, j, :],
                             start=(j == 0), stop=False)
            nc.tensor.matmul(zp[0:64, 1, :], W["fc2T"][:, j, 128:192], zr[:, j, :],
                             start=False, stop=(j == 1))
        nc.scalar.activation(Z1[g][:], zp[:, 0, :], AF.Identity, bias=W["fc2b1"][:])
        nc.scalar.activation(St[g][64:128, :], zp[0:64, 1, :], AF.Identity,
                             bias=W["fc2b2"][:])

    # ---- software pipeline: the two groups run a half-step out of phase.
    # Emission order per t:
    #   B1(g0,t) | A(g1,t) | B2(g0,t) | B1(g1,t) | prefetch | A(g0,t+1) | B2(g1,t)
    # Every dependency (A->B1->B2->A') crosses at least one chunk of the other
    # group's work, so no engine queue head-of-line-blocks on its producer.
    load_x(0)
    if t_steps > 1:
        load_x(1)
    GPs = {0: x_mms(0)}
    emit_A(0, 0, GPs[0])

    import os
    FENCE = int(os.environ.get("KFENCE", "9"))
    for t in range(t_steps):
        xts.pop(t, None)
        b0 = emit_B1(0, t) if FENCE >= 2 else None
        emit_A(1, t, GPs[t])
        if FENCE >= 3 and b0 is not None:
            emit_B2(0, t, *b0)
        b1 = emit_B1(1, t) if FENCE >= 2 else None
        if t + 2 < t_steps:
            load_x(t + 2)
        if t + 1 < t_steps:
            GPs[t + 1] = x_mms(t + 1)
            del GPs[t]
            emit_A(0, t + 1, GPs[t + 1])
        if FENCE >= 3 and b1 is not None:
            emit_B2(1, t, *b1)

    # ---------------- output head ----------------
    osb = tp.tile([1, B], F32, tag="osb")
    for g in range(G):
        o1p = pW[g].tile([64, BG], F32, tag="w")
        nc.tensor.matmul(o1p[:], W["o1T0"][:], Hl[g][:], start=True, stop=False)
        nc.tensor.matmul(o1p[:], W["o1T1"][:], St[g][:], start=False, stop=False)
        nc.tensor.matmul(o1p[:], W["o1T2"][:], Z1[g][:], start=False, stop=True)
        ro = tp.tile([64, BG], BF16, tag=f"ro{g}")
        nc.scalar.activation(ro[:], o1p[:], AF.Relu, bias=W["o1b"][:])
        o2p = pW[g].tile([1, BG], F32, tag="w")
        nc.tensor.matmul(o2p[:], W["o2T"][:], ro[:], start=True, stop=True)
        nc.scalar.activation(osb[0:1, gcols[g]], o2p[:], AF.Identity, bias=W["o2bt"][:])
    nc.sync.dma_start(out_d[:], osb[:])
    ctx.close()


# ------------------------------------------------------------------ driver
_NC_CACHE = {}


def make_in_maps(inputs):
    w = pack_weights(inputs)
    wb = {n: (v if n in F32_WEIGHTS else v.astype(BD)) for n, v in w.items()}
    x = np.asarray(inputs["x"], np.float32)
    t_steps = x.shape[0]
    in_maps = []
    for c in range(NCORES):
        xcT = x[:, c * B:(c + 1) * B, :].transpose(0, 2, 1)   # [T, D, B]
        xpk = np.zeros((t_steps, 128, 4, B), np.float32)
        xpk[:, :, 0, :] = xcT[:, 0:128]
        xpk[:, :, 1, :] = xcT[:, 128:256]
        xpk[:, 0:44, 2, :] = xcT[:, 256:300]
        xpk[:, 0:109, 3, :] = xcT[:, 300:409]
        m = {"xT": xpk.reshape(t_steps, 128, 4 * B).astype(BD)}
        m.update(wb)
        in_maps.append(m)
    return in_maps


def kernel(**inputs):
    x = np.asarray(inputs["x"], np.float32)
    t_steps = x.shape[0]
    key = t_steps
    if key not in _NC_CACHE:
        _NC_CACHE[key] = build_nc(t_steps)
    nc = _NC_CACHE[key]
    in_maps = make_in_maps(inputs)
    res = run_bass_kernel_spmd(nc, in_maps, list(range(NCORES)))
    out = np.empty((N, 1), np.float32)
    for c in range(NCORES):
        out[c * B:(c + 1) * B, 0] = res.results[c]["out"][0]
    return out


# revision 17
# speedup vs baseline: 1.0374x; 1.0374x over previous
"""MARN (multi-attention recurrent network) Trainium2 Bass kernel.

Strategy v2 (two-group software pipeline):
  - Data-parallel over batch N=2048 across 8 cores (256 rows each).
  - Feature-major on-chip layout: features on SBUF partitions, batch on the
    free dim.
  - The per-core batch of 256 is split into TWO independent groups of 128.
    The T-recurrence chain (gates -> acts -> cell -> attention -> softmax ->
    MLP -> z) is latency-bound; two groups give every engine a second
    independent stream to chew on while the other group's chain waits.
  - All matmul operands are bf16 (1 cyc/row at any moving size; fp32r would
    drop to 4 cyc/row below N=256). PSUM accumulation stays fp32.
  - Sigmoid via 0.5*tanh(0.5x)+0.5 so everything stays on the single
    `exp_and_others` ACT table (no 2.7us table reloads). States scaled 2x
    (C=2c, H=2h) with the 0.5 folded into consuming weights host-side.
  - Softmax over the feature dim via ones-matmuls; exp(att_b) folded into
    sum + reduction weights.
  - x-projections (no recurrence dependency) run at N=256 over both groups
    into a shared gate psum, prefetched one step ahead to fill PE bubbles.
"""
import sys

if "/opt/trn_rl_repo" not in sys.path:
    sys.path.insert(0, "/opt/trn_rl_repo")

import numpy as np
import ml_dtypes

import concourse.bass as bass
import concourse.bacc as bacc
import concourse.tile as tile
from concourse import mybir
from concourse.bass_utils import run_bass_kernel_spmd

F32 = mybir.dt.float32
F32R = mybir.dt.float32r
BF16 = mybir.dt.bfloat16
AF = mybir.ActivationFunctionType
ALU = mybir.AluOpType
BD = ml_dtypes.bfloat16

D_L, D_A, D_V = 300, 74, 35
DH_L, DH_A, DH_V = 128, 32, 32
H = DH_L + DH_A + DH_V          # 192
K = 4
RL, RA, RV = 32, 16, 16
RED = RL + RA + RV              # 64
MAP_H = 256
H_OUT = 64
T, N = 128, 2048
D = D_L + D_A + D_V             # 409
NCORES = 8
B = N // NCORES                 # 256 per-core batch
G = 2
BG = B // G                     # 128 per group


def r(x):
    return x.bitcast(F32R)


# ----------------------------------------------------------------- host pack
def pack_weights(i):
    """Pack reference weights into the on-chip layouts. Returns dict name->np."""
    w = {}
    f32 = np.float32

    # ---- L cell ----
    WlT = i["Wl_w"].T.astype(f32)                      # [300, 512]
    wlT = np.zeros((3, 128, 4 * DH_L), f32)
    wlT.reshape(384, 512)[:300] = WlT
    w["wlT"] = wlT
    w["ulT"] = (0.5 * i["Ul_w"].T).astype(f32)         # [128, 512]
    VlT = i["Vl_w"].T.astype(f32)                      # [192, 512]
    w["vlT1"] = VlT[:128].copy()
    vlSt = np.zeros((128, 512), f32)
    vlSt[64:128] = VlT[128:192]
    w["vlSt"] = vlSt
    bl = (i["Wl_b"] + i["Ul_b"] + i["Vl_b"]).astype(f32)   # [512]
    blb = np.empty((128, 4), f32)
    for m in range(3):
        blb[:, m] = 0.5 * bl[m * 128:(m + 1) * 128]
    blb[:, 3] = bl[384:512]
    w["blb"] = blb

    # ---- A/V cells fused: gate chunk m in {f,i,o,g}, cols [a(32) | v(32)] ----
    Wa, Wv = i["Wa_w"].astype(f32), i["Wv_w"].astype(f32)   # [128,74],[128,35]
    Ua, Uv = i["Ua_w"].astype(f32), i["Uv_w"].astype(f32)   # [128,32]
    Va, Vv = i["Va_w"].astype(f32), i["Vv_w"].astype(f32)   # [128,192]
    GATES = ((0, 3), (2, 1))          # (lo-half gate, hi-half gate) per chunk
    wavT = np.zeros((128, 2, 128), f32)
    uavSt = np.zeros((128, 2, 128), f32)
    vavT1 = np.zeros((128, 2, 128), f32)
    for m, pair in enumerate(GATES):
        for h, g in enumerate(pair):
            ga = slice(g * 32, (g + 1) * 32)
            c0 = h * 64
            wavT[0:74, m, c0:c0 + 32] = Wa[ga].T
            wavT[74:109, m, c0 + 32:c0 + 64] = Wv[ga].T
            uavSt[0:32, m, c0:c0 + 32] = 0.5 * Ua[ga].T
            uavSt[32:64, m, c0 + 32:c0 + 64] = 0.5 * Uv[ga].T
            uavSt[64:128, m, c0:c0 + 32] = Va[ga, 128:192].T
            uavSt[64:128, m, c0 + 32:c0 + 64] = Vv[ga, 128:192].T
            vavT1[:, m, c0:c0 + 32] = Va[ga, 0:128].T
            vavT1[:, m, c0 + 32:c0 + 64] = Vv[ga, 0:128].T
    w["wavT"], w["uavSt"], w["vavT1"] = wavT, uavSt, vavT1
    ba = (i["Wa_b"] + i["Ua_b"] + i["Va_b"]).astype(f32)
    bv = (i["Wv_b"] + i["Uv_b"] + i["Vv_b"]).astype(f32)
    bav = np.empty((128, 2), f32)
    for m, pair in enumerate(GATES):
        for h, g in enumerate(pair):
            s = 0.5 if g < 3 else 1.0
            bav[h * 64:h * 64 + 32, m] = s * ba[g * 32:(g + 1) * 32]
            bav[h * 64 + 32:h * 64 + 64, m] = s * bv[g * 32:(g + 1) * 32]
    w["bav"] = bav

    # ---- attention ----
    attT = i["att_w"].T.astype(f32)                    # [192, 768] cols (k,h')
    attT1 = np.empty((128, 4, 192), f32)
    attT2 = np.empty((64, 4, 192), f32)
    for k in range(4):
        blk = 0.5 * attT[:, k * 192:(k + 1) * 192]     # 0.5: C=2c fold
        attT1[:, k, :] = blk[0:128]
        attT2[:, k, :] = blk[128:192]
    w["attT1"], w["attT2"] = attT1, attT2
    eb = np.exp(i["att_b"].astype(np.float64)).astype(f32)  # [768]
    expb1 = np.stack([eb[k * 192:k * 192 + 128] for k in range(4)], 1)   # [128,4]
    expb2 = np.stack([eb[k * 192 + 128:(k + 1) * 192] for k in range(4)], 1)  # [64,4]

    ebZ1 = np.zeros((128, 4, 4), f32)
    ebZ2 = np.zeros((64, 4, 4), f32)
    for k in range(4):
        ebZ1[:, k, k] = expb1[:, k]
        ebZ2[:, k, k] = expb2[:, k]
    w["expbZ1"], w["expbZ2"] = ebZ1, ebZ2
    selm1 = np.zeros((4, 4, 128), f32)
    for k in range(4):
        selm1[k, k, :] = 1.0
    w["selm1"] = selm1

    # ---- reductions, with 0.5*exp(att_b) folded in ----
    rl = i["rl_w"].astype(f32)                         # [32, 512] cols (k,h')
    rlT = np.empty((128, 4, 32), f32)
    for k in range(4):
        scale = 0.5 * eb[k * 192:k * 192 + 128]        # [128]
        rlT[:, k, :] = rl[:, k * 128:(k + 1) * 128].T * scale[:, None]
    w["rlT"] = rlT
    ra = i["ra_w"].astype(f32)                         # [16, 128]
    rv = i["rv_w"].astype(f32)                         # [16, 128]
    ravT = np.zeros((64, 4, 64), f32)
    for k in range(4):
        sa = 0.5 * eb[k * 192 + 128:k * 192 + 160]     # [32]
        sv = 0.5 * eb[k * 192 + 160:k * 192 + 192]     # [32]
        ravT[0:32, k, 32:48] = ra[:, k * 32:(k + 1) * 32].T * sa[:, None]
        ravT[32:64, k, 48:64] = rv[:, k * 32:(k + 1) * 32].T * sv[:, None]
    w["ravT"] = ravT
    w["rbias"] = np.concatenate([i["rl_b"], i["ra_b"], i["rv_b"]]).astype(f32)[:, None]  # [64,1]

    # ---- z MLP ----
    w["fc1T"] = i["fc1_w"].T.astype(f32)               # [64, 256]
    fc1b = np.empty((128, 2), f32)
    fc1b[:, 0] = i["fc1_b"][0:128]
    fc1b[:, 1] = i["fc1_b"][128:256]
    w["fc1b"] = fc1b
    fc2T = np.empty((128, 2, 192), f32)
    fc2wT = i["fc2_w"].T.astype(f32)                   # [256, 192]
    fc2T[:, 0, :] = fc2wT[0:128]
    fc2T[:, 1, :] = fc2wT[128:256]
    w["fc2T"] = fc2T
    w["fc2b1"] = i["fc2_b"].astype(f32)[0:128, None]   # [128,1]
    w["fc2b2"] = i["fc2_b"].astype(f32)[128:192, None]  # [64,1]

    # ---- output head (h parts scaled 0.5 to consume H=2h) ----
    o1T = i["o1_w"].T.astype(f32)                      # [384, 64]
    w["o1T0"] = (0.5 * o1T[0:128]).copy()
    o1T1 = np.empty((128, 64), f32)
    o1T1[0:64] = 0.5 * o1T[128:192]                    # H_av
    o1T1[64:128] = o1T[320:384]                        # z2
    w["o1T1"] = o1T1
    w["o1T2"] = o1T[192:320].copy()                    # z1
    w["o1b"] = i["o1_b"].astype(f32)[:, None]          # [64,1]
    w["o2T"] = i["o2_w"].T.astype(f32)                 # [64, 1]
    w["o2bt"] = np.asarray(i["o2_b"], f32).reshape(1, 1)
    return w


F32_WEIGHTS = {"blb", "bav", "rbias", "fc1b", "fc2b1", "fc2b2", "o1b", "o2bt"}

WEIGHT_SHAPES = {
    "wlT": (3, 128, 512), "ulT": (128, 512), "vlT1": (128, 512),
    "vlSt": (128, 512), "blb": (128, 4),
    "wavT": (128, 2, 128), "uavSt": (128, 2, 128), "vavT1": (128, 2, 128),
    "bav": (128, 2),
    "attT1": (128, 4, 192), "attT2": (64, 4, 192),
    "expbZ1": (128, 4, 4), "expbZ2": (64, 4, 4),
    "selm1": (4, 4, 128),
    "rlT": (128, 4, 32), "ravT": (64, 4, 64), "rbias": (64, 1),
    "fc1T": (64, 256), "fc1b": (128, 2),
    "fc2T": (128, 2, 192), "fc2b1": (128, 1), "fc2b2": (64, 1),
    "o1T0": (128, 64), "o1T1": (128, 64), "o1T2": (128, 64),
    "o1b": (64, 1), "o2T": (64, 1), "o2bt": (1, 1),
}


# Weights ship as TWO flat dram tensors (one bf16, one f32) to minimize the
# per-execution buffer-binding count. Each entry is stored host-side already
# in its on-chip [partition, free] layout, flattened row-major.
def _flat_order():
    b16 = [n for n in WEIGHT_SHAPES if n not in F32_WEIGHTS]
    f32 = [n for n in WEIGHT_SHAPES if n in F32_WEIGHTS]
    return b16, f32


def _flat_offsets(names):
    offs, off = {}, 0
    for n in names:
        sz = int(np.prod(WEIGHT_SHAPES[n]))
        offs[n] = (off, sz)
        off += sz
    return offs, off


# ------------------------------------------------------------------ program
def build_nc(t_steps=T, x_bufs=3):
    nc = bacc.Bacc("TRN2", target_bir_lowering=False, debug=False,
                   num_devices=NCORES)
    # x pre-packed host-side to the SBUF tile layout [128 part, 4 slots, B]:
    # slots 0/1 = x_l[0:128]/[128:256], slot 2 rows 0:44 = x_l[256:300],
    # slot 3 rows 0:109 = x_a|x_v. One contiguous 2KB-per-partition DMA/step.
    xd = nc.dram_tensor("xT", [t_steps, 128, 4 * B], BF16, kind="ExternalInput").ap()
    out_d = nc.dram_tensor("out", [1, B], F32, kind="ExternalOutput").ap()
    b16n, f32n = _flat_order()
    ob, szb = _flat_offsets(b16n)
    of, szf = _flat_offsets(f32n)
    wflat = nc.dram_tensor("wflat", [szb], BF16, kind="ExternalInput").ap()
    bflat = nc.dram_tensor("bflat", [szf], F32, kind="ExternalInput").ap()
    # host stores every weight in its tile layout (wlT pre-transposed to
    # [128,3,512]); partition dim = first dim of the tile shape.
    wd = {}
    for n in b16n:
        off, sz = ob[n]
        p = 128 if n == "wlT" else WEIGHT_SHAPES[n][0]
        wd[n] = wflat[off:off + sz].rearrange("(p f) -> p f", p=p)
    for n in f32n:
        off, sz = of[n]
        wd[n] = bflat[off:off + sz].rearrange("(p f) -> p f", p=WEIGHT_SHAPES[n][0])

    with nc.allow_low_precision(reason="bf16 operand kernel; psum accumulation "
                                "remains fp32"):
        with tile.TileContext(nc) as tc:
            _emit(tc, xd, out_d, wd, t_steps, x_bufs)
    nc.compile()
    return nc


def _emit(tc, xd, out_d, wd, t_steps, x_bufs):
    nc = tc.nc
    import contextlib
    ctx = contextlib.ExitStack()

    wp = ctx.enter_context(tc.tile_pool(name="weights", bufs=1))
    sp = ctx.enter_context(tc.tile_pool(name="state", bufs=1))
    xp = ctx.enter_context(tc.tile_pool(name="x", bufs=x_bufs))
    tp = ctx.enter_context(tc.tile_pool(name="work", bufs=2))
    ep = ctx.enter_context(tc.tile_pool(name="exps", bufs=2))
    # PSUM: pGATE [128,6,256]f32 = 6KB = banks 0-2 (shared, bufs=1);
    # per-group work rings 2 x 2KB-slots = 1+1 banks each.
    pGATE = ctx.enter_context(tc.tile_pool(name="pGATE", bufs=1, space="PSUM"))
    pW = [ctx.enter_context(tc.tile_pool(name=f"pW{g}", bufs=2, space="PSUM"))
          for g in range(G)]

    # ---- persistent weights ----
    W = {}
    for n, shape in WEIGHT_SHAPES.items():
        tl = wp.tile(list(shape) if n != "wlT" else [128, 3, 512],
                     F32 if n in F32_WEIGHTS else BF16, tag=n, name=f"w_{n}")
        nc.sync.dma_start(tl[:], wd[n])
        W[n] = tl

    # ---- per-group states (in-place updated each step) ----
    Hl, St, Z1, Cl, Cav = [], [], [], [], []
    for g in range(G):
        Hl.append(sp.tile([128, BG], BF16, tag=f"Hl{g}", name=f"Hl{g}"))
        St.append(sp.tile([128, BG], BF16, tag=f"St{g}", name=f"St{g}"))
        Z1.append(sp.tile([128, BG], BF16, tag=f"Z1{g}", name=f"Z1{g}"))
        Cl.append(sp.tile([128, BG], BF16, tag=f"Cl{g}", name=f"Cl{g}"))
        Cav.append(sp.tile([64, BG], BF16, tag=f"Cav{g}", name=f"Cav{g}"))
        for s in (Hl[g], St[g], Z1[g], Cl[g]):
            nc.vector.memset(s[:].bitcast(F32), 0.0)
        nc.vector.memset(Cav[g][:].bitcast(F32), 0.0)

    xts = {}

    def load_x(t):
        xt = xp.tile([128, 4, B], BF16, tag="xt")
        nc.sync.dma_start(xt[:], xd[t, :, :])
        xts[t] = xt

    def x_mms(t):
        """Input projections for step t at N=256 (both groups), into a fresh
        shared gate psum GP [128, 6, 256]: slots 0-3 L gate chunks (banks 0-1),
        slots 4-5 AV chunks (bank 2). One start per bank."""
        GP = pGATE.tile([128, 6, B], F32, tag="gate")
        xt = xts[t]
        for m in range(4):
            for j in range(3):
                kk = 128 if j < 2 else 44
                nc.tensor.matmul(GP[:, m, :], W["wlT"][0:kk, j, m * 128:(m + 1) * 128],
                                 xt[0:kk, j, :], start=(j == 0 and m % 2 == 0),
                                 stop=False)
        for m in range(2):
            nc.tensor.matmul(GP[:, 4 + m, :], W["wavT"][0:109, m, :], xt[0:109, 3, :],
                             start=(m == 0), stop=False)
        return GP

    gcols = [slice(g * BG, (g + 1) * BG) for g in range(G)]

    import os
    AFENCE = int(os.environ.get("KAFENCE", "9"))

    def emit_A(g, t, GP):
        """Gate matmuls + activations + cell updates for group g, step t."""
        gc = gcols[g]
        if AFENCE < 1:
            return
        for m in range(4):
            nc.tensor.matmul(GP[:, m, gc], W["ulT"][:, m * 128:(m + 1) * 128],
                             Hl[g][:], start=False, stop=False)
        for m in range(4):
            ms = slice(m * 128, (m + 1) * 128)
            nc.tensor.matmul(GP[:, m, gc], W["vlT1"][:, ms], Z1[g][:],
                             start=False, stop=False)
            nc.tensor.matmul(GP[:, m, gc], W["vlSt"][:, ms], St[g][:],
                             start=False, stop=(g == G - 1 and m % 2 == 1))
        for m in range(2):
            nc.tensor.matmul(GP[:, 4 + m, gc], W["uavSt"][:, m, :], St[g][:],
                             start=False, stop=False)
            nc.tensor.matmul(GP[:, 4 + m, gc], W["vavT1"][:, m, :], Z1[g][:],
                             start=False, stop=(g == G - 1 and m == 1))
        if AFENCE < 2:
            return
        # gate activations: tf/ti/to = tanh(0.5 s + 0.5 b), tg = tanh(s + b)
        tf = tp.tile([128, 4, BG], BF16, tag=f"tfl{g}", name=f"tfl{g}")
        for m in range(4):
            sc = 0.5 if m < 3 else 1.0
            nc.scalar.activation(tf[:, m, :], GP[:, m, gc], AF.Tanh,
                                 bias=W["blb"][:, m:m + 1], scale=sc)
        ta = tp.tile([128, 2, BG], BF16, tag=f"tav{g}", name=f"tav{g}")
        nc.scalar.activation(ta[0:64, 0, :], GP[0:64, 4, gc], AF.Tanh,
                             bias=W["bav"][0:64, 0:1], scale=0.5)
        nc.scalar.activation(ta[64:128, 0, :], GP[64:128, 4, gc], AF.Tanh,
                             bias=W["bav"][64:128, 0:1], scale=1.0)
        nc.scalar.activation(ta[:, 1, :], GP[:, 5, gc], AF.Tanh,
                             bias=W["bav"][:, 1:2], scale=0.5)
        if AFENCE < 3:
            return
        # cell updates (C=2c, H=2h)
        s1l = tp.tile([128, BG], BF16, tag=f"s1l{g}", name=f"s1l{g}")
        s2l = tp.tile([128, BG], BF16, tag=f"s2l{g}", name=f"s2l{g}")
        nc.vector.scalar_tensor_tensor(s1l[:], tf[:, 0, :], 1.0, Cl[g][:], ALU.add, ALU.mult)
        nc.vector.scalar_tensor_tensor(s2l[:], tf[:, 1, :], 1.0, tf[:, 3, :], ALU.add, ALU.mult)
        nc.vector.scalar_tensor_tensor(Cl[g][:], s1l[:], 0.5, s2l[:], ALU.mult, ALU.add)
        s1a = tp.tile([128, BG], BF16, tag=f"s1a{g}", name=f"s1a{g}")
        s2a = tp.tile([128, BG], BF16, tag=f"s2a{g}", name=f"s2a{g}")
        nc.vector.scalar_tensor_tensor(s1a[64:128, :], ta[0:64, 0, :], 1.0, Cav[g][:], ALU.add, ALU.mult)
        nc.vector.scalar_tensor_tensor(s2a[64:128, :], ta[64:128, 1, :], 1.0, ta[64:128, 0, :], ALU.add, ALU.mult)
        nc.vector.scalar_tensor_tensor(Cav[g][:], s1a[64:128, :], 0.5, s2a[64:128, :], ALU.mult, ALU.add)
        tcl = tp.tile([128, BG], BF16, tag=f"tcl{g}", name=f"tcl{g}")
        tca = tp.tile([64, BG], BF16, tag=f"tca{g}", name=f"tca{g}")
        nc.scalar.activation(tcl[:], Cl[g][:], AF.Tanh, scale=0.5)
        nc.scalar.activation(tca[:], Cav[g][:], AF.Tanh, scale=0.5)
        nc.vector.scalar_tensor_tensor(Hl[g][:], tf[:, 2, :], 1.0, tcl[:], ALU.add, ALU.mult)
        nc.vector.scalar_tensor_tensor(St[g][0:64, :], ta[0:64, 1, :], 1.0, tca[:], ALU.add, ALU.mult)

    def emit_B1(g, t):
        """Attention matmuls, exp, softmax sums + reciprocal for group g."""
        att1 = pW[g].tile([128, 4, BG], F32, tag="w", name=f"att1_{g}")
        att2 = pW[g].tile([64, 4, BG], F32, tag="w", name=f"att2_{g}")
        for k in range(4):
            nc.tensor.matmul(att1[:, k, :], W["attT1"][:, k, 0:128], Cl[g][:],
                             start=(k == 0), stop=False)
            nc.tensor.matmul(att1[:, k, :], W["attT2"][:, k, 0:128], Cav[g][:],
                             start=False, stop=(k == 3))
            nc.tensor.matmul(att2[:, k, :], W["attT1"][:, k, 128:192], Cl[g][:],
                             start=(k == 0), stop=False)
            nc.tensor.matmul(att2[:, k, :], W["attT2"][:, k, 128:192], Cav[g][:],
                             start=False, stop=(k == 3))
        e1 = ep.tile([128, 4, BG], BF16, tag=f"e1{g}", name=f"e1{g}")
        e2 = ep.tile([64, 4, BG], BF16, tag=f"e2{g}", name=f"e2{g}")
        nc.scalar.activation(e1[:], att1[:], AF.Exp)
        nc.scalar.activation(e2[:], att2[:], AF.Exp)
        S4 = pW[g].tile([4, BG], F32, tag="w", name=f"S4_{g}")
        for ki in range(8):
            k, side = divmod(ki, 2)
            lh = W["expbZ1"][:, k, :] if side == 0 else W["expbZ2"][:, k, :]
            rh = e1[:, k, :] if side == 0 else e2[:, k, :]
            nc.tensor.matmul(S4[0:4, :], lh, rh, start=(ki == 0), stop=(ki == 7))
        rs4 = tp.tile([4, BG], BF16, tag=f"rs{g}", name=f"rs{g}")
        nc.vector.reciprocal(rs4[0:4, :], S4[0:4, :].bitcast(F32R))
        return e1, e2, rs4

    def emit_B2(g, t, e1, e2, rs4):
        """Broadcast 1/S, scale attended, reductions + z MLP for group g."""
        rb1 = pW[g].tile([128, 4, BG], F32, tag="w", name=f"rb1_{g}")
        for k in range(4):
            nc.tensor.matmul(rb1[:, k, :], W["selm1"][:, k, :], rs4[0:4, :],
                             start=(k == 0), stop=(k == 3))
        for k in range(4):
            nc.vector.tensor_tensor(e1[:, k, :], e1[:, k, :], Cl[g][:], ALU.mult)
            nc.vector.tensor_tensor(e1[:, k, :], e1[:, k, :],
                                    rb1[:, k, :].bitcast(F32R), ALU.mult)
            nc.vector.tensor_tensor(e2[:, k, :], e2[:, k, :], Cav[g][:], ALU.mult)
            nc.vector.tensor_tensor(e2[:, k, :], e2[:, k, :],
                                    rb1[0:64, k, :].bitcast(F32R), ALU.mult)
        redp = pW[g].tile([64, BG], F32, tag="w", name=f"redp_{g}")
        for k in range(3):
            nc.tensor.matmul(redp[0:64, :], W["ravT"][:, k, :], e2[:, k, :],
                             start=(k == 0), stop=False)
        for k in range(4):
            nc.tensor.matmul(redp[0:32, :], W["rlT"][:, k, :], e1[:, k, :],
                             start=False, stop=False)
        nc.tensor.matmul(redp[0:64, :], W["ravT"][:, 3, :], e2[:, 3, :],
                         start=False, stop=True)
        rsb = tp.tile([64, BG], BF16, tag=f"rsb{g}", name=f"rsb{g}")
        nc.scalar.activation(rsb[:], redp[:], AF.Identity, bias=W["rbias"][:])
        f1p = pW[g].tile([128, 2, BG], F32, tag="w", name=f"f1p_{g}")
        for m in range(2):
            nc.tensor.matmul(f1p[:, m, :], W["fc1T"][:, m * 128:(m + 1) * 128],
                             rsb[:], start=(m == 0), stop=(m == 1))
        zr = tp.tile([128, 2, BG], BF16, tag=f"zr{g}", name=f"zr{g}")
        for m in range(2):
            nc.scalar.activation(zr[:, m, :], f1p[:, m, :], AF.Relu,
                                 bias=W["fc1b"][:, m:m + 1])
        zp = pW[g].tile([128, 2, BG], F32, tag="w", name=f"zp_{g}")
        for j in range(2):
            nc.tensor.matmul(zp[:, 0, :], W["fc2T"][:, j, 0:128], zr[:, j, :],
                             start=(j == 0), stop=False)
            nc.tensor.matmul(zp[0:64, 1, :], W["fc2T"][:, j, 128:192], zr[:, j, :],
                             start=False, stop=(j == 1))
        nc.scalar.activation(Z1[g][:], zp[:, 0, :], AF.Identity, bias=W["fc2b1"][:])
        nc.scalar.activation(St[g][64:128, :], zp[0:64, 1, :], AF.Identity,
                             bias=W["fc2b2"][:])

    # ---- software pipeline: the two groups run a half-step out of phase.
    # Emission order per t:
    #   B1(g0,t) | A(g1,t) | B2(g0,t) | B1(g1,t) | prefetch | A(g0,t+1) | B2(g1,t)
    # Every dependency (A->B1->B2->A') crosses at least one chunk of the other
    # group's work, so no engine queue head-of-line-blocks on its producer.
    load_x(0)
    if t_steps > 1:
        load_x(1)
    GPs = {0: x_mms(0)}
    emit_A(0, 0, GPs[0])

    import os
    FENCE = int(os.environ.get("KFENCE", "9"))
    for t in range(t_steps):
        xts.pop(t, None)
        b0 = emit_B1(0, t) if FENCE >= 2 else None
        emit_A(1, t, GPs[t])
        if FENCE >= 3 and b0 is not None:
            emit_B2(0, t, *b0)
        b1 = emit_B1(1, t) if FENCE >= 2 else None
        if t + 2 < t_steps:
            load_x(t + 2)
        if t + 1 < t_steps:
            GPs[t + 1] = x_mms(t + 1)
            del GPs[t]
            emit_A(0, t + 1, GPs[t + 1])
        if FENCE >= 3 and b1 is not None:
            emit_B2(1, t, *b1)

    # ---------------- output head ----------------
    osb = tp.tile([1, B], F32, tag="osb")
    for g in range(G):
        o1p = pW[g].tile([64, BG], F32, tag="w")
        nc.tensor.matmul(o1p[:], W["o1T0"][:], Hl[g][:], start=True, stop=False)
        nc.tensor.matmul(o1p[:], W["o1T1"][:], St[g][:], start=False, stop=False)
        nc.tensor.matmul(o1p[:], W["o1T2"][:], Z1[g][:], start=False, stop=True)
        ro = tp.tile([64, BG], BF16, tag=f"ro{g}")
        nc.scalar.activation(ro[:], o1p[:], AF.Relu, bias=W["o1b"][:])
        o2p = pW[g].tile([1, BG], F32, tag="w")
        nc.tensor.matmul(o2p[:], W["o2T"][:], ro[:], start=True, stop=True)
        nc.scalar.activation(osb[0:1, gcols[g]], o2p[:], AF.Identity, bias=W["o2bt"][:])
    nc.sync.dma_start(out_d[:], osb[:])
    ctx.close()


# ------------------------------------------------------------------ driver
_NC_CACHE = {}


def make_in_maps(inputs):
    w = pack_weights(inputs)
    w["wlT"] = np.ascontiguousarray(w["wlT"].transpose(1, 0, 2))  # [128,3,512]
    b16n, f32n = _flat_order()
    wflat = np.concatenate([np.asarray(w[n], np.float32).reshape(-1)
                            for n in b16n]).astype(BD)
    bflat = np.concatenate([np.asarray(w[n], np.float32).reshape(-1)
                            for n in f32n])
    x = np.asarray(inputs["x"], np.float32)
    t_steps = x.shape[0]
    in_maps = []
    for c in range(NCORES):
        xcT = x[:, c * B:(c + 1) * B, :].transpose(0, 2, 1)   # [T, D, B]
        xpk = np.zeros((t_steps, 128, 4, B), np.float32)
        xpk[:, :, 0, :] = xcT[:, 0:128]
        xpk[:, :, 1, :] = xcT[:, 128:256]
        xpk[:, 0:44, 2, :] = xcT[:, 256:300]
        xpk[:, 0:109, 3, :] = xcT[:, 300:409]
        m = {"xT": xpk.reshape(t_steps, 128, 4 * B).astype(BD),
             "wflat": wflat, "bflat": bflat}
        in_maps.append(m)
    return in_maps


def kernel(**inputs):
    x = np.asarray(inputs["x"], np.float32)
    t_steps = x.shape[0]
    key = t_steps
    if key not in _NC_CACHE:
        _NC_CACHE[key] = build_nc(t_steps)
    nc = _NC_CACHE[key]
    in_maps = make_in_maps(inputs)
    res = run_bass_kernel_spmd(nc, in_maps, list(range(NCORES)))
    out = np.empty((N, 1), np.float32)
    for c in range(NCORES):
        out[c * B:(c + 1) * B, 0] = res.results[c]["out"][0]
    return out
